# revision 30
# baseline (speedup 1.0000x reference)
"""Trainium2 kernel for nn_Dense_Q_MulIn1Out_Conv1D.

The reference "quantum conv" circuit is linear in the state vector: three
RY-rotation layers interleaved with a fixed 512x512 orthogonal entangler.
The circuit collapses to one orthogonal matrix M (512x512); with the encoded
state nonzero only in its first 128 amplitudes, the <Z> readout is a
quadratic form with a fixed symmetric 128x128 matrix A = Md^T Z Md
(Md = M[:, :128], orthonormal columns; Z = diag(+-1)):

    out[n] = (v_n^T A v_n) / (||v_n||^2 + 1e-12)

Eigendecomposition A = Q diag(lam) Q^T (Q orthogonal, |lam| <= 1) gives a
second formulation: with U = Q^T V,

    num[n] = sum_j lam_j U[j,n]^2        den[n] = sum_j U[j,n]^2

Device plan (per core, 2 of 16 batches), fp16 data path:
  - x pre-cast to fp16 on host; im2col V [128, 4096] per batch via c-major
    DMAs (outer AP dim = 16 channels -> 16 descriptors spread over all 16
    DMA engines; pieces kept <= 1024 cols so every packet is <= 2KB). x
    pieces on the sync HWDGE queue, consts concurrently on the scalar one.
  - Junk-data warmup matmuls (no DMA dependency, junk lands in red and is
    reset by the first accumulation start) bridge the PE from kernel start
    to the first real matmul so the DVFS clock ramp starts early.
  - Per chunk pair: two U = Q^T V matmuls (fp16, fp32 PSUM) + one 1024-col
    Scalar ACTIVATE Square (amortizes per-op overhead; DVE cannot dual-read
    PSUM, so Scalar owns the square stream and paces the kernel).
  - Reduce per chunk g: 16-wide moving-window selector matmuls - lam-column
    at PE col group q0 -> num lands at PSUM partition g (base 0), ones
    column at q32 -> den at partition 32+g. Adjacent q0/q32 matmuls run
    CONCURRENTLY on the PE, so a chunk's full reduce costs ~512 col-times.
  - Tail (two pipelined 256-col halves): Scalar copy of den (nonzero-base
    PSUM), Vector reciprocal + multiply (num read directly from base-0
    PSUM), fp16 output DMA on both HWDGE queues.
"""

import numpy as np

_DIM = 512
_D = 128
_K = 8
_C = 16
_NQ = 9
_B = 16
_L = 4096
_L_OUT = _L - _K + 1  # 4089
_N_CORES = 8
_B_PER_CORE = _B // _N_CORES  # 2
_CHUNK = 512
_NCHUNK = 8  # 512-col chunks per batch
_NG = _B_PER_CORE * _NCHUNK  # 16 chunks per core
_LV = 4096
_NWARM = 22
# consts columns: Q | T_lam (lam col at 143) | T_ones (ones col at 175) | A
_Q0 = 0
_TL0 = 128
_TO0 = 160
_A0 = 192
_NCONST = 320


def _apply_ry_layer(psi, angles):
    # psi [N, DIM] float64; matches reference._apply_ry_layer
    for q in range(_NQ):
        half = angles[q] * 0.5
        c, s = np.cos(half), np.sin(half)
        left = 2 ** q
        p = psi.reshape(-1, left, 2, _DIM // (2 ** (q + 1)))
        a, b = p[:, :, 0, :].copy(), p[:, :, 1, :].copy()
        psi = np.stack([c * a - s * b, s * a + c * b], axis=2).reshape(-1, _DIM)
    return psi


def _build_mats(entangle_matrix, theta):
    """Collapse the circuit to A (c-major patch order) and its eigenbasis."""
    U = np.asarray(entangle_matrix, dtype=np.float64)
    th = np.asarray(theta, dtype=np.float64)
    psi = np.eye(_DIM, dtype=np.float64)
    for l in range(th.shape[0]):
        psi = _apply_ry_layer(psi, th[l])
        psi = psi @ U.T
    M = psi.T  # state map: s -> M s
    z = np.concatenate([np.ones(_DIM // 2), -np.ones(_DIM // 2)])
    Md = M[:, :_D]
    A = Md.T @ (z[:, None] * Md)
    lam, Q = np.linalg.eigh(A)
    return A, lam, Q


_NC_CACHE = {}


def _build_nc():
    import concourse.tile as tile
    from concourse import bacc, mybir
    from bass_rust import AP as RawAP

    F16 = mybir.dt.float16
    F32 = mybir.dt.float32
    AF = mybir.ActivationFunctionType

    nc = bacc.Bacc(
        "TRN2",
        target_bir_lowering=False,
        debug=False,
        num_devices=_N_CORES,
    )
    # flat fp16 x for this core's 2 batches + 64 pad elements so the im2col
    # window never reads out of bounds
    x = nc.dram_tensor(
        "x", [_B_PER_CORE * _C * _L + 64], F16, kind="ExternalInput"
    ).ap()
    consts = nc.dram_tensor("consts", [_D * _NCONST], F16, kind="ExternalInput").ap()
    out = nc.dram_tensor("out", [_NG, _CHUNK], F16, kind="ExternalOutput").ap()

    with tile.TileContext(nc) as tc:
        from contextlib import ExitStack

        with ExitStack() as ctx:
            const_pool = ctx.enter_context(tc.tile_pool(name="const", bufs=1))
            v_pool = ctx.enter_context(tc.tile_pool(name="v", bufs=2))
            p_pool = ctx.enter_context(tc.tile_pool(name="p", bufs=5))
            u_pool = ctx.enter_context(tc.tile_pool(name="u", bufs=3, space="PSUM"))
            red_pool = ctx.enter_context(tc.tile_pool(name="red", bufs=1, space="PSUM"))
            o_pool = ctx.enter_context(tc.tile_pool(name="o", bufs=1))

            # Warmup burst on memset junk (no DMA dependency): keeps the PE
            # busy from kernel start until the first V piece lands, so the
            # clock ramp begins as early as possible.
            w_sb = const_pool.tile([_D, _D], F16, tag="wsb")
            nc.vector.memset(w_sb[:], 1.0)

            c_sb = const_pool.tile([_D, _NCONST], F16)

            def const_piece(c0, w):
                # flat consts with a 16-outer AP: 16 descriptors, fast expansion
                cap = RawAP(
                    tensor=consts.tensor,
                    offset=c0,
                    ap=[[_NCONST * 8, 16], [_NCONST, 8], [1, w]],
                )
                nc.scalar.dma_start(c_sb[:, c0 : c0 + w], cap)

            q_sb = c_sb[:, _Q0 : _Q0 + _D]
            a_sb = c_sb[:, _A0 : _A0 + _D]

            def sel_num(g):
                # 16-wide window: lam-column lands at within-window col g
                return c_sb[:, _TL0 + 15 - g : _TL0 + 31 - g]

            def sel_den(g):
                return c_sb[:, _TO0 + 15 - g : _TO0 + 31 - g]

            # im2col, c-major: V[c*8+k, n] = x[b, c, n+k]. Outer AP dim = 16
            # channels -> descriptors fan out over all 16 DMA engines.
            vs = []
            for b in range(_B_PER_CORE):
                v = v_pool.tile([_D, _LV], F16, tag="v")
                vs.append(v)

            def piece(b, c0, w):
                srcap = RawAP(
                    tensor=x.tensor,
                    offset=b * _C * _L + c0,
                    ap=[[_L, _C], [1, _K], [1, w]],
                )
                nc.sync.dma_start(vs[b][:, c0 : c0 + w], srcap)

            # all pieces <= 1024 cols: keeps every DMA packet at <= 2KB
            # (4KB packets from 2048-col pieces drain ~2x slower per byte)
            const_piece(0, _NCONST)
            piece(0, 0, 1024)
            piece(0, 1024, 1024)
            piece(0, 2048, 1024)
            piece(0, 3072, 1024)
            piece(1, 0, 1024)
            piece(1, 1024, 1024)
            piece(1, 2048, 1024)
            piece(1, 3072, 1024)

            # red PSUM tile: num accumulates at partitions 0:16 (row g =
            # chunk g), den at 32:48. The warmup matmuls dump junk here
            # first - the reduces open with start=True, resetting the PSUM.
            red = red_pool.tile([_D, _CHUNK], F32)
            kw = dict(skip_group_check=True)
            for _ in range(_NWARM):
                nc.tensor.matmul(red[:, :_D], w_sb[:], w_sb[:],
                                 start=True, stop=True, **kw)

            def emit_reduces(blk):
                # num at q0 -> partition g (base 0, DVE-readable); den at
                # q32 -> partition 32+g. Adjacent q0/q32 matmuls execute
                # concurrently on the PE.
                for i in range(4):
                    g = 4 * blk + i
                    if g < 2:
                        num_sel, (num_rhs, den_rhs) = sel_den(g), p_of[g]
                    else:
                        num_sel = sel_num(g)
                        num_rhs = den_rhs = p_of[g]
                    nc.tensor.matmul(
                        red[0:16, :], num_sel, num_rhs,
                        tile_position=(0, 0),
                        start=(g == 0), stop=(g == _NG - 1), **kw,
                    )
                    nc.tensor.matmul(
                        red[32:48, :], sel_den(g), den_rhs,
                        tile_position=(0, 32),
                        start=(g == 0), stop=(g == _NG - 1), **kw,
                    )

            # mains per chunk, squares per chunk PAIR (1024-col ACTIVATE
            # amortizes the ~230ns per-op Scalar overhead)
            p_of = {}
            for blk in range(4):
                for half in range(2):
                    g = 4 * blk + 2 * half
                    b, c0 = g // _NCHUNK, (g % _NCHUNK) * _CHUNK
                    vp = vs[b][:, c0 : c0 + 2 * _CHUNK]
                    u = u_pool.tile([_D, 2 * _CHUNK], F32, tag="u")
                    p = p_pool.tile([_D, 2 * _CHUNK], F16, tag="p")
                    lhs = a_sb if g == 0 else q_sb
                    for q in range(2):
                        nc.tensor.matmul(
                            u[:, q * _CHUNK : (q + 1) * _CHUNK], lhs,
                            vs[b][:, c0 + q * _CHUNK : c0 + (q + 1) * _CHUNK],
                            start=True, stop=True,
                        )
                        if g == 2:
                            # split this pair's square so the Scalar stream
                            # starts right after main 2, not main 3
                            nc.scalar.activation(
                                p[:, q * _CHUNK : (q + 1) * _CHUNK],
                                u[:, q * _CHUNK : (q + 1) * _CHUNK], AF.Square,
                            )
                    if g == 0:
                        # chunks 0,1 ride the quadratic path on the (idle)
                        # Vector engine: Y = A V in u; p1 = V.Y, p2 = V.V.
                        # Scalar's square stream starts one pair later but
                        # is one pair shorter.
                        p2 = p_pool.tile([_D, 2 * _CHUNK], F16, tag="p")
                        nc.vector.tensor_mul(p[:], vp, u[:])
                        nc.vector.tensor_mul(p2[:], vp, vp)
                        p_of[0] = (p[:, 0:_CHUNK], p2[:, 0:_CHUNK])
                        p_of[1] = (p[:, _CHUNK:], p2[:, _CHUNK:])
                        continue
                    if g != 2:
                        nc.scalar.activation(p[:], u[:], AF.Square)
                    p_of[g] = p[:, 0:_CHUNK]
                    p_of[g + 1] = p[:, _CHUNK : 2 * _CHUNK]
                if blk >= 1:
                    emit_reduces(blk - 1)
            emit_reduces(3)

            # Tail, pipelined in two 256-col halves with per-half tiles
            # (shared tiles would add false WAR deps that serialize them).
            den0 = o_pool.tile([16, 256], F32, tag="den0")
            den1 = o_pool.tile([16, 256], F32, tag="den1")
            rden0 = o_pool.tile([16, 256], F32, tag="rden0")
            rden1 = o_pool.tile([16, 256], F32, tag="rden1")
            out0 = o_pool.tile([16, 256], F16, tag="out0")
            out1 = o_pool.tile([16, 256], F16, tag="out1")
            halves = [slice(0, 256), slice(256, 512)]
            dens, rdens, outs_sb = [den0, den1], [rden0, rden1], [out0, out1]
            for h, sl in enumerate(halves):
                # den partitions 32:48 are not DVE-readable; stage via Scalar
                nc.scalar.activation(dens[h][:], red[32:48, sl], AF.Copy)
            for h, sl in enumerate(halves):
                nc.vector.reciprocal_approx_fast(rdens[h][:], dens[h][:])
                nc.vector.tensor_mul(outs_sb[h][:], red[0:16, sl], rdens[h][:])
                q = nc.sync if h == 0 else nc.scalar
                q.dma_start(out[:, sl], outs_sb[h][:])

    nc.compile()
    return nc


def get_nc():
    if "nc" not in _NC_CACHE:
        _NC_CACHE["nc"] = _build_nc()
    return _NC_CACHE["nc"]


def kernel(x, entangle_matrix, theta, _trace=False, **trace_kwargs):
    from concourse.bass_utils import run_bass_kernel_spmd

    x16 = np.asarray(x).astype(np.float16)
    _A, lam, Q = _build_mats(entangle_matrix, theta)  # A used for chunks 0-1
    consts = np.zeros((_D, _NCONST), dtype=np.float16)
    consts[:, _Q0 : _Q0 + _D] = Q.astype(np.float16)
    consts[:, _A0 : _A0 + _D] = _A.astype(np.float16)
    consts[:, _TL0 + 15] = lam.astype(np.float16)
    consts[:, _TO0 + 15] = 1.0

    nc = get_nc()
    pad = np.zeros(64, dtype=np.float16)
    in_maps = [
        {
            "x": np.concatenate(
                [x16[i * _B_PER_CORE : (i + 1) * _B_PER_CORE].reshape(-1), pad]
            ),
            "consts": consts.reshape(-1),
        }
        for i in range(_N_CORES)
    ]
    res = run_bass_kernel_spmd(
        nc, in_maps, list(range(_N_CORES)), trace=_trace, **trace_kwargs
    )
    outs = []
    for i in range(_N_CORES):
        o = np.asarray(res.results[i]["out"], dtype=np.float32)
        # row g = batch (g//8), col block (g%8)
        outs.append(o.reshape(_B_PER_CORE, _NCHUNK * _CHUNK)[:, :_L_OUT])
    full = np.concatenate(outs, axis=0).reshape(_B, 1, 1, _L_OUT)
    if _trace:
        kernel._last_results = res
    return full


# revision 31
# speedup vs baseline: 1.0759x; 1.0759x over previous
"""Trainium2 kernel for nn_Dense_Q_MulIn1Out_Conv1D.

The reference "quantum conv" circuit is linear in the state vector: three
RY-rotation layers interleaved with a fixed 512x512 orthogonal entangler.
The circuit collapses to one orthogonal matrix M (512x512); with the encoded
state nonzero only in its first 128 amplitudes, the <Z> readout is a
quadratic form with a fixed symmetric 128x128 matrix A = Md^T Z Md
(Md = M[:, :128], orthonormal columns; Z = diag(+-1)):

    out[n] = (v_n^T A v_n) / (||v_n||^2 + 1e-12)

Eigendecomposition A = Q diag(lam) Q^T (Q orthogonal, |lam| <= 1) gives a
second formulation: with U = Q^T V,

    num[n] = sum_j lam_j U[j,n]^2        den[n] = sum_j U[j,n]^2

Device plan (per core, 2 of 16 batches), fp16 data path:
  - x pre-cast to fp16 on host; im2col V [128, 4096] per batch via c-major
    DMAs (outer AP dim = 16 channels -> 16 descriptors spread over all 16
    DMA engines; pieces kept <= 1024 cols so every packet is <= 2KB). x
    pieces on the sync HWDGE queue, consts concurrently on the scalar one.
  - Junk-data warmup matmuls (no DMA dependency, junk lands in red and is
    reset by the first accumulation start) bridge the PE from kernel start
    to the first real matmul so the DVFS clock ramp starts early.
  - Per chunk pair: two U = Q^T V matmuls (fp16, fp32 PSUM) + one 1024-col
    Scalar ACTIVATE Square (amortizes per-op overhead; DVE cannot dual-read
    PSUM, so Scalar owns the square stream and paces the kernel).
  - Reduce per chunk g: 16-wide moving-window selector matmuls - lam-column
    at PE col group q0 -> num lands at PSUM partition g (base 0), ones
    column at q32 -> den at partition 32+g. Adjacent q0/q32 matmuls run
    CONCURRENTLY on the PE, so a chunk's full reduce costs ~512 col-times.
  - Tail (two pipelined 256-col halves): Scalar copy of den (nonzero-base
    PSUM), Vector reciprocal + multiply (num read directly from base-0
    PSUM), fp16 output DMA on both HWDGE queues.
"""

import numpy as np

_DIM = 512
_D = 128
_K = 8
_C = 16
_NQ = 9
_B = 16
_L = 4096
_L_OUT = _L - _K + 1  # 4089
_N_CORES = 8
_B_PER_CORE = _B // _N_CORES  # 2
_CHUNK = 512
_NCHUNK = 8  # 512-col chunks per batch
_NG = _B_PER_CORE * _NCHUNK  # 16 chunks per core
_LV = 4096
_NWARM = 22
# consts columns: Q | T_lam (lam col at 143) | T_ones (ones col at 175)
_Q0 = 0
_TL0 = 128
_TO0 = 160
_NCONST = 192


def _apply_ry_layer(psi, angles):
    # psi [N, DIM] float64; matches reference._apply_ry_layer
    for q in range(_NQ):
        half = angles[q] * 0.5
        c, s = np.cos(half), np.sin(half)
        left = 2 ** q
        p = psi.reshape(-1, left, 2, _DIM // (2 ** (q + 1)))
        a, b = p[:, :, 0, :].copy(), p[:, :, 1, :].copy()
        psi = np.stack([c * a - s * b, s * a + c * b], axis=2).reshape(-1, _DIM)
    return psi


def _build_mats(entangle_matrix, theta):
    """Collapse the circuit to A (c-major patch order) and its eigenbasis."""
    U = np.asarray(entangle_matrix, dtype=np.float64)
    th = np.asarray(theta, dtype=np.float64)
    psi = np.eye(_DIM, dtype=np.float64)
    for l in range(th.shape[0]):
        psi = _apply_ry_layer(psi, th[l])
        psi = psi @ U.T
    M = psi.T  # state map: s -> M s
    z = np.concatenate([np.ones(_DIM // 2), -np.ones(_DIM // 2)])
    Md = M[:, :_D]
    A = Md.T @ (z[:, None] * Md)
    lam, Q = np.linalg.eigh(A)
    return A, lam, Q


_NC_CACHE = {}


def _build_nc():
    import concourse.tile as tile
    from concourse import bacc, mybir
    from bass_rust import AP as RawAP

    F16 = mybir.dt.float16
    F32 = mybir.dt.float32
    AF = mybir.ActivationFunctionType

    nc = bacc.Bacc(
        "TRN2",
        target_bir_lowering=False,
        debug=False,
        num_devices=_N_CORES,
    )
    # flat fp16 x for this core's 2 batches + 64 pad elements so the im2col
    # window never reads out of bounds
    x = nc.dram_tensor(
        "x", [_B_PER_CORE * _C * _L + 64], F16, kind="ExternalInput"
    ).ap()
    consts = nc.dram_tensor("consts", [_D * _NCONST], F16, kind="ExternalInput").ap()
    out = nc.dram_tensor("out", [_NG, _CHUNK], F16, kind="ExternalOutput").ap()

    with tile.TileContext(nc) as tc:
        from contextlib import ExitStack

        with ExitStack() as ctx:
            const_pool = ctx.enter_context(tc.tile_pool(name="const", bufs=1))
            v_pool = ctx.enter_context(tc.tile_pool(name="v", bufs=2))
            p_pool = ctx.enter_context(tc.tile_pool(name="p", bufs=5))
            u_pool = ctx.enter_context(tc.tile_pool(name="u", bufs=3, space="PSUM"))
            red_pool = ctx.enter_context(tc.tile_pool(name="red", bufs=1, space="PSUM"))
            o_pool = ctx.enter_context(tc.tile_pool(name="o", bufs=1))

            # Warmup burst on memset junk (no DMA dependency): keeps the PE
            # busy from kernel start until the first V piece lands, so the
            # clock ramp begins as early as possible.
            w_sb = const_pool.tile([_D, _D], F16, tag="wsb")
            nc.vector.memset(w_sb[:], 1.0)

            c_sb = const_pool.tile([_D, _NCONST], F16)

            def const_piece(c0, w):
                # flat consts with a 16-outer AP: 16 descriptors, fast expansion
                cap = RawAP(
                    tensor=consts.tensor,
                    offset=c0,
                    ap=[[_NCONST * 8, 16], [_NCONST, 8], [1, w]],
                )
                nc.scalar.dma_start(c_sb[:, c0 : c0 + w], cap)

            q_sb = c_sb[:, _Q0 : _Q0 + _D]

            def sel_num(g):
                # 16-wide window: lam-column lands at within-window col g
                return c_sb[:, _TL0 + 15 - g : _TL0 + 31 - g]

            def sel_den(g):
                return c_sb[:, _TO0 + 15 - g : _TO0 + 31 - g]

            # im2col, c-major: V[c*8+k, n] = x[b, c, n+k]. Outer AP dim = 16
            # channels -> descriptors fan out over all 16 DMA engines.
            vs = []
            for b in range(_B_PER_CORE):
                v = v_pool.tile([_D, _LV], F16, tag="v")
                vs.append(v)

            def piece(b, c0, w):
                srcap = RawAP(
                    tensor=x.tensor,
                    offset=b * _C * _L + c0,
                    ap=[[_L, _C], [1, _K], [1, w]],
                )
                nc.sync.dma_start(vs[b][:, c0 : c0 + w], srcap)

            # all pieces <= 1024 cols: keeps every DMA packet at <= 2KB
            # (4KB packets from 2048-col pieces drain ~2x slower per byte)
            const_piece(0, _NCONST)
            piece(0, 0, 1024)
            piece(0, 1024, 1024)
            piece(0, 2048, 1024)
            piece(0, 3072, 1024)
            piece(1, 0, 1024)
            piece(1, 1024, 1024)
            piece(1, 2048, 1024)
            piece(1, 3072, 1024)

            # red PSUM tile: num accumulates at partitions 0:16 (row g =
            # chunk g), den at 32:48. The warmup matmuls dump junk here
            # first - the reduces open with start=True, resetting the PSUM.
            red = red_pool.tile([_D, _CHUNK], F32)
            kw = dict(skip_group_check=True)
            for _ in range(_NWARM):
                nc.tensor.matmul(red[:, :_D], w_sb[:], w_sb[:],
                                 start=True, stop=True, **kw)

            def emit_reduces(blk):
                # num at q0 -> partition g (base 0, DVE-readable); den at
                # q32 -> partition 32+g. Adjacent q0/q32 matmuls execute
                # concurrently on the PE.
                for i in range(4):
                    g = 4 * blk + i
                    nc.tensor.matmul(
                        red[0:16, :], sel_num(g), p_of[g],
                        tile_position=(0, 0),
                        start=(g == 0), stop=(g == _NG - 1), **kw,
                    )
                    nc.tensor.matmul(
                        red[32:48, :], sel_den(g), p_of[g],
                        tile_position=(0, 32),
                        start=(g == 0), stop=(g == _NG - 1), **kw,
                    )

            # mains per chunk, squares per chunk PAIR (1024-col ACTIVATE
            # amortizes the ~230ns per-op Scalar overhead)
            p_of = {}
            for blk in range(4):
                for half in range(2):
                    g = 4 * blk + 2 * half
                    b, c0 = g // _NCHUNK, (g % _NCHUNK) * _CHUNK
                    u = u_pool.tile([_D, 2 * _CHUNK], F32, tag="u")
                    p = p_pool.tile([_D, 2 * _CHUNK], F16, tag="p")
                    for q in range(2):
                        nc.tensor.matmul(
                            u[:, q * _CHUNK : (q + 1) * _CHUNK], q_sb,
                            vs[b][:, c0 + q * _CHUNK : c0 + (q + 1) * _CHUNK],
                            start=True, stop=True,
                        )
                        if g == 0:
                            # split the first pair's square so the Scalar
                            # stream starts right after main 0, not main 1
                            nc.scalar.activation(
                                p[:, q * _CHUNK : (q + 1) * _CHUNK],
                                u[:, q * _CHUNK : (q + 1) * _CHUNK], AF.Square,
                            )
                    if g != 0:
                        nc.scalar.activation(p[:], u[:], AF.Square)
                    p_of[g] = p[:, 0:_CHUNK]
                    p_of[g + 1] = p[:, _CHUNK : 2 * _CHUNK]
                if blk >= 1:
                    emit_reduces(blk - 1)
            emit_reduces(3)

            # Tail, pipelined in two 256-col halves with per-half tiles
            # (shared tiles would add false WAR deps that serialize them).
            den0 = o_pool.tile([16, 256], F32, tag="den0")
            den1 = o_pool.tile([16, 256], F32, tag="den1")
            rden0 = o_pool.tile([16, 256], F32, tag="rden0")
            rden1 = o_pool.tile([16, 256], F32, tag="rden1")
            out0 = o_pool.tile([16, 256], F16, tag="out0")
            out1 = o_pool.tile([16, 256], F16, tag="out1")
            halves = [slice(0, 256), slice(256, 512)]
            dens, rdens, outs_sb = [den0, den1], [rden0, rden1], [out0, out1]
            for h, sl in enumerate(halves):
                # den partitions 32:48 are not DVE-readable; stage via Scalar
                nc.scalar.activation(dens[h][:], red[32:48, sl], AF.Copy)
            for h, sl in enumerate(halves):
                nc.vector.reciprocal_approx_fast(rdens[h][:], dens[h][:])
                nc.vector.tensor_mul(outs_sb[h][:], red[0:16, sl], rdens[h][:])
                q = nc.sync if h == 0 else nc.scalar
                q.dma_start(out[:, sl], outs_sb[h][:])

    nc.compile()
    return nc


def get_nc():
    if "nc" not in _NC_CACHE:
        _NC_CACHE["nc"] = _build_nc()
    return _NC_CACHE["nc"]


def kernel(x, entangle_matrix, theta, _trace=False, **trace_kwargs):
    from concourse.bass_utils import run_bass_kernel_spmd

    x16 = np.asarray(x).astype(np.float16)
    _A, lam, Q = _build_mats(entangle_matrix, theta)
    consts = np.zeros((_D, _NCONST), dtype=np.float16)
    consts[:, _Q0 : _Q0 + _D] = Q.astype(np.float16)
    consts[:, _TL0 + 15] = lam.astype(np.float16)
    consts[:, _TO0 + 15] = 1.0

    nc = get_nc()
    pad = np.zeros(64, dtype=np.float16)
    in_maps = [
        {
            "x": np.concatenate(
                [x16[i * _B_PER_CORE : (i + 1) * _B_PER_CORE].reshape(-1), pad]
            ),
            "consts": consts.reshape(-1),
        }
        for i in range(_N_CORES)
    ]
    res = run_bass_kernel_spmd(
        nc, in_maps, list(range(_N_CORES)), trace=_trace, **trace_kwargs
    )
    outs = []
    for i in range(_N_CORES):
        o = np.asarray(res.results[i]["out"], dtype=np.float32)
        # row g = batch (g//8), col block (g%8)
        outs.append(o.reshape(_B_PER_CORE, _NCHUNK * _CHUNK)[:, :_L_OUT])
    full = np.concatenate(outs, axis=0).reshape(_B, 1, 1, _L_OUT)
    if _trace:
        kernel._last_results = res
    return full
